# revision 3
# baseline (speedup 1.0000x reference)
"""Distributed attention kernel for 8 TRN2 NeuronCores.

Reference computation (torch-Linear convention, x @ W.T + b):
    qp = q @ Wq.T + bq ; kp = k @ Wk.T + bk ; vp = v @ Wv.T + bv
    weights    = qp @ kp.T                  [N, N]
    normalized = softmax(weights, -1)       [N, N]  (returned)
    out        = (normalized @ vp) @ Wo.T + bo      (returned)

Sharding: rows (N) of q across the 8 cores.  Each core projects its own
row-shard of q/k/v; the projected kp^T (fp16) and vp (bf16) shards are
all-gathered so every core holds the full [H, N] kp^T and [N, H] vp
resident in SBUF.  Each core then computes its 1024-row block of the
attention matrix, softmax (no max-subtraction needed: |logits| < 40 and
exp accumulates in fp32), the normalized output rows, and the final
projection.

Precision: logits are computed with fp16 matmuls (measured max logit
error ~1e-2 -> ~1% worst-case softmax-entry error), projections with
float32r (~5e-3 max logit error), PV / output projection in bf16.
exp-weights are held in bf16 (fp16 would overflow: exp(logit) ~ 1e13).
"""
import sys

sys.path.insert(0, "/opt/trn_rl_repo")

import numpy as np

import concourse.bacc as bacc
import concourse.mybir as mybir
from concourse import masks, tile
from concourse.bass_utils import run_bass_kernel_spmd

N = 8192
H = 512
C = 8
NL = N // C  # 1024 rows per core
NCH = NL // 128  # 8 row-chunks per core
SB = 16  # s-blocks of 512
HCH = H // 128  # 4

F32 = mybir.dt.float32
F32R = mybir.dt.float32r
BF16 = mybir.dt.bfloat16
FP16 = mybir.dt.float16

LAST_RESULT = None  # BassKernelResults of the most recent run (for profiling)


def _build():
    nc = bacc.Bacc("TRN2", target_bir_lowering=False, debug=False, num_devices=C)

    q = nc.dram_tensor("q", [NL, H], F32, kind="ExternalInput")
    k = nc.dram_tensor("k", [NL, H], F32, kind="ExternalInput")
    v = nc.dram_tensor("v", [NL, H], F32, kind="ExternalInput")
    wqt = nc.dram_tensor("wqt", [H, H], F32R, kind="ExternalInput")
    wkt = nc.dram_tensor("wkt", [H, H], F32R, kind="ExternalInput")
    wvt = nc.dram_tensor("wvt", [H, H], F32R, kind="ExternalInput")
    wot = nc.dram_tensor("wot", [H, H], F32, kind="ExternalInput")
    bq = nc.dram_tensor("bq", [HCH, 128, 1], F32, kind="ExternalInput")
    bk = nc.dram_tensor("bk", [HCH, 128, 1], F32, kind="ExternalInput")
    bvb = nc.dram_tensor("bvb", [128, H], F32, kind="ExternalInput")
    bob = nc.dram_tensor("bob", [128, H], F32, kind="ExternalInput")

    norm_o = nc.dram_tensor("norm", [NL, N], F32, kind="ExternalOutput")
    out_o = nc.dram_tensor("o", [NL, H], F32, kind="ExternalOutput")

    # collective buffers (internal DRAM)
    kpt_sh = nc.dram_tensor("kpt_sh", [H, NL], FP16)
    vp_sh = nc.dram_tensor("vp_sh", [NL, H], BF16)
    kpt_g = nc.dram_tensor("kpt_g", [C, H, NL], FP16, addr_space="Shared")
    vp_g = nc.dram_tensor("vp_g", [N, H], BF16, addr_space="Shared")

    with tile.TileContext(nc) as tc:
        with (
            tc.tile_pool(name="const", bufs=1) as constp,
            tc.tile_pool(name="qpt", bufs=1) as qptp,
        ):
            ident_f32 = constp.tile([128, 128], F32)
            ident_bf16 = constp.tile([128, 128], BF16)
            masks.make_identity(nc, ident_f32[:])
            masks.make_identity(nc, ident_bf16[:])
            bq_t = constp.tile([128, HCH], F32)
            bk_t = constp.tile([128, HCH], F32)
            nc.sync.dma_start(out=bq_t[:], in_=bq.rearrange("j p one -> p (j one)"))
            nc.sync.dma_start(out=bk_t[:], in_=bk.rearrange("j p one -> p (j one)"))
            bvb_t = constp.tile([128, H], F32)
            bob_t = constp.tile([128, H], F32)
            nc.sync.dma_start(out=bvb_t[:], in_=bvb[:])
            nc.sync.dma_start(out=bob_t[:], in_=bob[:])
            wot_f = constp.tile([128, HCH, H], F32)
            nc.sync.dma_start(out=wot_f[:], in_=wot.rearrange("(j p) o -> p j o", p=128))
            wot_t = constp.tile([128, HCH, H], BF16)
            nc.vector.tensor_copy(wot_t[:], wot_f[:])

            qpt_t = qptp.tile([128, HCH, NL], FP16)  # qp^T resident

            # ---------------- Phase A: transposes + projections ----------
            with (
                tc.tile_pool(name="wabc", bufs=1) as wp,
                tc.tile_pool(name="raw", bufs=3) as rawp,
                tc.tile_pool(name="tT", bufs=2) as tTp,
                tc.tile_pool(name="shard", bufs=1) as shp,
                tc.tile_pool(name="ps_tr", bufs=2, space="PSUM") as ps_tr,
                tc.tile_pool(name="ps_pr", bufs=2, space="PSUM") as ps_pr,
            ):
                w_tiles = {}
                for name, wt in (("wq", wqt), ("wk", wkt), ("wv", wvt)):
                    w_t = wp.tile([128, HCH, H], F32R, tag="wall")
                    nc.sync.dma_start(
                        out=w_t[:], in_=wt.rearrange("(j p) o -> p j o", p=128)
                    )
                    w_tiles[name] = w_t

                kpsh_t = shp.tile([128, HCH, NL], FP16)
                vpsh_t = shp.tile([128, NCH, H], BF16)

                for ti, (src, wname) in enumerate(
                    ((q, "wq"), (k, "wk"), (v, "wv"))
                ):
                    # transpose the [NL, H] shard into tT [h, n] (f32r)
                    tT_t = tTp.tile([128, HCH, NL], F32R, tag="tT")
                    for r in range(NCH):
                        raw_t = rawp.tile([128, H], F32, tag="raw")
                        nc.sync.dma_start(
                            out=raw_t[:], in_=src[r * 128 : (r + 1) * 128, :]
                        )
                        ptr = ps_tr.tile([128, HCH, 128], F32, tag="ptr")
                        for j in range(HCH):
                            nc.tensor.transpose(
                                ptr[:, j, :],
                                raw_t[:, j * 128 : (j + 1) * 128],
                                ident_f32[:],
                            )
                        nc.vector.tensor_copy(
                            tT_t[:, :, r * 128 : (r + 1) * 128], ptr[:]
                        )
                    w_t = w_tiles[wname]
                    if ti == 0:  # q -> qp^T (fp16), bias per-partition
                        for m in range(HCH):
                            for nb in range(NL // 512):
                                pp = ps_pr.tile([128, 512], F32, tag="pp")
                                for j in range(HCH):
                                    nc.tensor.matmul(
                                        pp[:],
                                        w_t[:, j, m * 128 : (m + 1) * 128],
                                        tT_t[:, j, nb * 512 : (nb + 1) * 512],
                                        start=(j == 0),
                                        stop=(j == HCH - 1),
                                    )
                                nc.scalar.activation(
                                    qpt_t[:, m, nb * 512 : (nb + 1) * 512],
                                    pp[:],
                                    mybir.ActivationFunctionType.Identity,
                                    bias=bq_t[:, m : m + 1],
                                    scale=1.0,
                                )
                    elif ti == 1:  # k -> kp^T shard (fp16)
                        for m in range(HCH):
                            for nb in range(NL // 512):
                                pp = ps_pr.tile([128, 512], F32, tag="pp")
                                for j in range(HCH):
                                    nc.tensor.matmul(
                                        pp[:],
                                        w_t[:, j, m * 128 : (m + 1) * 128],
                                        tT_t[:, j, nb * 512 : (nb + 1) * 512],
                                        start=(j == 0),
                                        stop=(j == HCH - 1),
                                    )
                                nc.scalar.activation(
                                    kpsh_t[:, m, nb * 512 : (nb + 1) * 512],
                                    pp[:],
                                    mybir.ActivationFunctionType.Identity,
                                    bias=bk_t[:, m : m + 1],
                                    scale=1.0,
                                )
                    else:  # v -> vp shard (bf16), bias along free dim
                        for r in range(NCH):
                            pp = ps_pr.tile([128, 512], F32, tag="pp")
                            for j in range(HCH):
                                nc.tensor.matmul(
                                    pp[:],
                                    tT_t[:, j, r * 128 : (r + 1) * 128],
                                    w_t[:, j, :],
                                    start=(j == 0),
                                    stop=(j == HCH - 1),
                                )
                            nc.vector.tensor_add(vpsh_t[:, r, :], pp[:], bvb_t[:])

                # ship shards to DRAM and all-gather
                nc.sync.dma_start(
                    out=kpt_sh.rearrange("(m p) s -> p m s", p=128), in_=kpsh_t[:]
                )
                nc.sync.dma_start(
                    out=vp_sh.rearrange("(r p) o -> p r o", p=128), in_=vpsh_t[:]
                )
                nc.gpsimd.collective_compute(
                    "AllGather",
                    mybir.AluOpType.bypass,
                    replica_groups=[list(range(C))],
                    ins=[kpt_sh[:].opt()],
                    outs=[kpt_g[:].opt()],
                )
                nc.gpsimd.collective_compute(
                    "AllGather",
                    mybir.AluOpType.bypass,
                    replica_groups=[list(range(C))],
                    ins=[vp_sh[:].opt()],
                    outs=[vp_g[:].opt()],
                )

            # ---------------- Phase C/D: attention ----------------------
            with (
                tc.tile_pool(name="kres", bufs=1) as kresp,
                tc.tile_pool(name="vres", bufs=1) as vresp,
                tc.tile_pool(name="expw", bufs=2) as expwp,
                tc.tile_pool(name="wstage", bufs=2) as wstp,
                tc.tile_pool(name="estage", bufs=3) as estp,
                tc.tile_pool(name="stats", bufs=2) as statp,
                tc.tile_pool(name="osmall", bufs=1) as osp,
                tc.tile_pool(name="ps_w", bufs=2, space="PSUM") as ps_w,
                tc.tile_pool(name="ps_t", bufs=2, space="PSUM") as ps_t,
                tc.tile_pool(name="ps_o", bufs=2, space="PSUM") as ps_o,
                tc.tile_pool(name="ps_m", bufs=1, space="PSUM") as ps_m,
            ):
                kres_t = kresp.tile([128, HCH, N], FP16)
                for c in range(C):
                    nc.sync.dma_start(
                        out=kres_t[:, :, c * NL : (c + 1) * NL],
                        in_=kpt_g[c].rearrange("(j p) s -> p j s", p=128),
                    )
                vres_t = vresp.tile([128, N // 128, H], BF16)
                for c in range(C):
                    nc.sync.dma_start(
                        out=vres_t[:, c * (NL // 128) : (c + 1) * (NL // 128), :],
                        in_=vp_g[c * NL : (c + 1) * NL].rearrange(
                            "(sc p) o -> p sc o", p=128
                        ),
                    )

                for i in range(NCH):
                    expw_t = expwp.tile([128, N], BF16, tag="expw")
                    rs_t = statp.tile([128, SB], F32, tag="rs")
                    po = ps_o.tile([128, H], F32, tag="po")
                    for sb in range(SB):
                        pw = ps_w.tile([128, 512], F32, tag="pw")
                        for j in range(HCH):
                            nc.tensor.matmul(
                                pw[:],
                                qpt_t[:, j, i * 128 : (i + 1) * 128],
                                kres_t[:, j, sb * 512 : (sb + 1) * 512],
                                start=(j == 0),
                                stop=(j == HCH - 1),
                            )
                        nc.scalar.activation(
                            expw_t[:, sb * 512 : (sb + 1) * 512],
                            pw[:],
                            mybir.ActivationFunctionType.Exp,
                            accum_out=rs_t[:, sb : sb + 1],
                        )
                        pt = ps_t.tile([128, 512], BF16, tag="pt")
                        for sc in range(4):
                            nc.tensor.transpose(
                                pt[:, sc * 128 : (sc + 1) * 128],
                                expw_t[:, sb * 512 + sc * 128 : sb * 512 + (sc + 1) * 128],
                                ident_bf16[:],
                            )
                        et = estp.tile([128, 512], BF16, tag="et")
                        nc.vector.tensor_copy(et[:], pt[:])
                        for sc in range(4):
                            nc.tensor.matmul(
                                po[:],
                                et[:, sc * 128 : (sc + 1) * 128],
                                vres_t[:, sb * 4 + sc, :],
                                start=(sb == 0 and sc == 0),
                                stop=(sb == SB - 1 and sc == 3),
                            )
                    rsum_t = statp.tile([128, 1], F32, tag="rsum")
                    nc.vector.reduce_sum(
                        rsum_t[:], rs_t[:], axis=mybir.AxisListType.X
                    )
                    recip_t = statp.tile([128, 1], F32, tag="recip")
                    nc.vector.reciprocal(recip_t[:], rsum_t[:])

                    # normalized rows out (overlaps next chunk's compute)
                    for wb in range(8):
                        wst = wstp.tile([128, 1024], F32, tag="wst")
                        nc.scalar.activation(
                            wst[:],
                            expw_t[:, wb * 1024 : (wb + 1) * 1024],
                            mybir.ActivationFunctionType.Copy,
                            scale=recip_t[:],
                        )
                        nc.sync.dma_start(
                            out=norm_o[
                                i * 128 : (i + 1) * 128, wb * 1024 : (wb + 1) * 1024
                            ],
                            in_=wst[:],
                        )

                    # out rows: scale, transpose, project, bias
                    osc_t = osp.tile([128, H], BF16, tag="osc")
                    nc.scalar.activation(
                        osc_t[:],
                        po[:],
                        mybir.ActivationFunctionType.Copy,
                        scale=recip_t[:],
                    )
                    pt2 = ps_m.tile([128, HCH, 128], BF16, tag="pt2")
                    for j in range(HCH):
                        nc.tensor.transpose(
                            pt2[:, j, :],
                            osc_t[:, j * 128 : (j + 1) * 128],
                            ident_bf16[:],
                        )
                    oT_t = osp.tile([128, HCH, 128], BF16, tag="oT")
                    nc.vector.tensor_copy(oT_t[:], pt2[:])
                    pf = ps_m.tile([128, H], F32, tag="pf")
                    for j in range(HCH):
                        nc.tensor.matmul(
                            pf[:],
                            oT_t[:, j, :],
                            wot_t[:, j, :],
                            start=(j == 0),
                            stop=(j == HCH - 1),
                        )
                    of_t = osp.tile([128, H], F32, tag="of")
                    nc.vector.tensor_add(of_t[:], pf[:], bob_t[:])
                    nc.sync.dma_start(
                        out=out_o[i * 128 : (i + 1) * 128, :], in_=of_t[:]
                    )

    nc.compile()
    return nc


_NC = None


def kernel(q, k, v, Wq, bq, Wk, bk, Wv, bv, Wo, bo):
    global _NC, LAST_RESULT
    q = np.ascontiguousarray(np.asarray(q, dtype=np.float32))
    k = np.ascontiguousarray(np.asarray(k, dtype=np.float32))
    v = np.ascontiguousarray(np.asarray(v, dtype=np.float32))
    Wq = np.asarray(Wq, dtype=np.float32)
    Wk = np.asarray(Wk, dtype=np.float32)
    Wv = np.asarray(Wv, dtype=np.float32)
    Wo = np.asarray(Wo, dtype=np.float32)
    bq = np.asarray(bq, dtype=np.float32)
    bk = np.asarray(bk, dtype=np.float32)
    bv = np.asarray(bv, dtype=np.float32)
    bo = np.asarray(bo, dtype=np.float32)

    if _NC is None:
        _NC = _build()

    wqt = np.ascontiguousarray(Wq.T)
    wkt = np.ascontiguousarray(Wk.T)
    wvt = np.ascontiguousarray(Wv.T)
    wot = np.ascontiguousarray(Wo.T)
    bq_r = np.ascontiguousarray(bq.reshape(HCH, 128, 1))
    bk_r = np.ascontiguousarray(bk.reshape(HCH, 128, 1))
    bvb = np.ascontiguousarray(np.broadcast_to(bv[None, :], (128, H)))
    bob = np.ascontiguousarray(np.broadcast_to(bo[None, :], (128, H)))

    in_maps = []
    for c in range(C):
        in_maps.append(
            {
                "q": q[c * NL : (c + 1) * NL],
                "k": k[c * NL : (c + 1) * NL],
                "v": v[c * NL : (c + 1) * NL],
                "wqt": wqt,
                "wkt": wkt,
                "wvt": wvt,
                "wot": wot,
                "bq": bq_r,
                "bk": bk_r,
                "bvb": bvb,
                "bob": bob,
            }
        )

    res = run_bass_kernel_spmd(_NC, in_maps, core_ids=list(range(C)))
    LAST_RESULT = res
    normalized = np.concatenate(
        [res.results[c]["norm"] for c in range(C)], axis=0
    )
    out = np.concatenate([res.results[c]["o"] for c in range(C)], axis=0)
    return (out, normalized)


# revision 6
# speedup vs baseline: 1.0384x; 1.0384x over previous
"""Distributed attention kernel for 8 TRN2 NeuronCores.

Reference computation (torch-Linear convention, x @ W.T + b):
    qp = q @ Wq.T + bq ; kp = k @ Wk.T + bk ; vp = v @ Wv.T + bv
    weights    = qp @ kp.T                  [N, N]
    normalized = softmax(weights, -1)       [N, N]  (returned)
    out        = (normalized @ vp) @ Wo.T + bo      (returned)

Sharding: rows (N) of q across the 8 cores.  Each core projects its own
row-shard of q/k/v; the projected kp^T (fp16) and vp (bf16) shards are
all-gathered so every core holds the full [H, N] kp^T and [N, H] vp
resident in SBUF.  Each core then computes its 1024-row block of the
attention matrix, softmax (no max-subtraction needed: |logits| < 40 and
exp accumulates in fp32), the normalized output rows, and the final
projection.

Precision: logits are computed with fp16 matmuls (measured max logit
error ~1e-2 -> ~1% worst-case softmax-entry error), projections with
float32r (~5e-3 max logit error), PV / output projection in bf16.
exp-weights are held in bf16 (fp16 would overflow: exp(logit) ~ 1e13).
"""
import sys

sys.path.insert(0, "/opt/trn_rl_repo")

import numpy as np

import concourse.bacc as bacc
import concourse.mybir as mybir
from concourse import masks, tile
from concourse.bass_utils import run_bass_kernel_spmd

N = 8192
H = 512
C = 8
NL = N // C  # 1024 rows per core
NCH = NL // 128  # 8 row-chunks per core
SB = 16  # s-blocks of 512
HCH = H // 128  # 4

F32 = mybir.dt.float32
F32R = mybir.dt.float32r
BF16 = mybir.dt.bfloat16
FP16 = mybir.dt.float16

LAST_RESULT = None  # BassKernelResults of the most recent run (for profiling)


def _build():
    nc = bacc.Bacc("TRN2", target_bir_lowering=False, debug=False, num_devices=C)

    q = nc.dram_tensor("q", [NL, H], F32, kind="ExternalInput")
    k = nc.dram_tensor("k", [NL, H], F32, kind="ExternalInput")
    v = nc.dram_tensor("v", [NL, H], F32, kind="ExternalInput")
    wqt = nc.dram_tensor("wqt", [H, H], F32R, kind="ExternalInput")
    wkt = nc.dram_tensor("wkt", [H, H], F32R, kind="ExternalInput")
    wvt = nc.dram_tensor("wvt", [H, H], F32R, kind="ExternalInput")
    wot = nc.dram_tensor("wot", [H, H], F32, kind="ExternalInput")
    bq = nc.dram_tensor("bq", [HCH, 128, 1], F32, kind="ExternalInput")
    bk = nc.dram_tensor("bk", [HCH, 128, 1], F32, kind="ExternalInput")
    bvb = nc.dram_tensor("bvb", [128, H], F32, kind="ExternalInput")
    bob = nc.dram_tensor("bob", [128, H], F32, kind="ExternalInput")

    norm_o = nc.dram_tensor("norm", [NL, N], F32, kind="ExternalOutput")
    out_o = nc.dram_tensor("o", [NL, H], F32, kind="ExternalOutput")

    # collective buffers (internal DRAM)
    kpt_sh = nc.dram_tensor("kpt_sh", [H, NL], FP16)
    vp_sh = nc.dram_tensor("vp_sh", [NL, H], BF16)
    kpt_g = nc.dram_tensor("kpt_g", [C, H, NL], FP16, addr_space="Shared")
    vp_g = nc.dram_tensor("vp_g", [N, H], BF16, addr_space="Shared")

    with tile.TileContext(nc) as tc:
        with (
            tc.tile_pool(name="const", bufs=1) as constp,
            tc.tile_pool(name="qpt", bufs=1) as qptp,
        ):
            ident_f32 = constp.tile([128, 128], F32)
            ident_bf16 = constp.tile([128, 128], BF16)
            masks.make_identity(nc, ident_f32[:])
            masks.make_identity(nc, ident_bf16[:])
            bq_t = constp.tile([128, HCH], F32)
            bk_t = constp.tile([128, HCH], F32)
            nc.sync.dma_start(out=bq_t[:], in_=bq.rearrange("j p one -> p (j one)"))
            nc.sync.dma_start(out=bk_t[:], in_=bk.rearrange("j p one -> p (j one)"))
            bvb_t = constp.tile([128, H], F32)
            bob_t = constp.tile([128, H], F32)
            nc.sync.dma_start(out=bvb_t[:], in_=bvb[:])
            nc.sync.dma_start(out=bob_t[:], in_=bob[:])
            wot_f = constp.tile([128, HCH, H], F32)
            nc.sync.dma_start(out=wot_f[:], in_=wot.rearrange("(j p) o -> p j o", p=128))
            wot_t = constp.tile([128, HCH, H], BF16)
            nc.vector.tensor_copy(wot_t[:], wot_f[:])

            qpt_t = qptp.tile([128, HCH, NL], FP16)  # qp^T resident

            # ---------------- Phase A: transposes + projections ----------
            with (
                tc.tile_pool(name="wabc", bufs=1) as wp,
                tc.tile_pool(name="raw", bufs=3) as rawp,
                tc.tile_pool(name="tT", bufs=2) as tTp,
                tc.tile_pool(name="shard", bufs=1) as shp,
                tc.tile_pool(name="ps_tr", bufs=2, space="PSUM") as ps_tr,
                tc.tile_pool(name="ps_pr", bufs=2, space="PSUM") as ps_pr,
            ):
                w_tiles = {}
                for name, wt in (("wq", wqt), ("wk", wkt), ("wv", wvt)):
                    w_t = wp.tile([128, HCH, H], F32R, tag=name)
                    nc.sync.dma_start(
                        out=w_t[:], in_=wt.rearrange("(j p) o -> p j o", p=128)
                    )
                    w_tiles[name] = w_t

                kpsh_t = shp.tile([128, HCH, NL], FP16)
                vpsh_t = shp.tile([128, NCH, H], BF16)

                for src, wname in ((k, "wk"), (v, "wv"), (q, "wq")):
                    # transpose the [NL, H] shard into tT [h, n] (f32r)
                    tT_t = tTp.tile([128, HCH, NL], F32R, tag="tT")
                    for r in range(NCH):
                        raw_t = rawp.tile([128, H], F32, tag="raw")
                        nc.sync.dma_start(
                            out=raw_t[:], in_=src[r * 128 : (r + 1) * 128, :]
                        )
                        ptr = ps_tr.tile([128, HCH, 128], F32, tag="ptr")
                        for j in range(HCH):
                            nc.tensor.transpose(
                                ptr[:, j, :],
                                raw_t[:, j * 128 : (j + 1) * 128],
                                ident_f32[:],
                            )
                        nc.vector.tensor_copy(
                            tT_t[:, :, r * 128 : (r + 1) * 128], ptr[:]
                        )
                    w_t = w_tiles[wname]
                    if wname == "wq":  # q -> qp^T (fp16), bias per-partition
                        for m in range(HCH):
                            for nb in range(NL // 512):
                                pp = ps_pr.tile([128, 512], F32, tag="pp")
                                for j in range(HCH):
                                    nc.tensor.matmul(
                                        pp[:],
                                        w_t[:, j, m * 128 : (m + 1) * 128],
                                        tT_t[:, j, nb * 512 : (nb + 1) * 512],
                                        start=(j == 0),
                                        stop=(j == HCH - 1),
                                    )
                                nc.scalar.activation(
                                    qpt_t[:, m, nb * 512 : (nb + 1) * 512],
                                    pp[:],
                                    mybir.ActivationFunctionType.Identity,
                                    bias=bq_t[:, m : m + 1],
                                    scale=1.0,
                                )
                    elif wname == "wk":  # k -> kp^T shard (fp16)
                        for m in range(HCH):
                            for nb in range(NL // 512):
                                pp = ps_pr.tile([128, 512], F32, tag="pp")
                                for j in range(HCH):
                                    nc.tensor.matmul(
                                        pp[:],
                                        w_t[:, j, m * 128 : (m + 1) * 128],
                                        tT_t[:, j, nb * 512 : (nb + 1) * 512],
                                        start=(j == 0),
                                        stop=(j == HCH - 1),
                                    )
                                nc.scalar.activation(
                                    kpsh_t[:, m, nb * 512 : (nb + 1) * 512],
                                    pp[:],
                                    mybir.ActivationFunctionType.Identity,
                                    bias=bk_t[:, m : m + 1],
                                    scale=1.0,
                                )
                        nc.sync.dma_start(
                            out=kpt_sh.rearrange("(m p) s -> p m s", p=128),
                            in_=kpsh_t[:],
                        )
                        nc.gpsimd.collective_compute(
                            "AllGather",
                            mybir.AluOpType.bypass,
                            replica_groups=[list(range(C))],
                            ins=[kpt_sh[:].opt()],
                            outs=[kpt_g[:].opt()],
                        )
                    else:  # v -> vp shard (bf16), bias along free dim
                        for r in range(NCH):
                            pp = ps_pr.tile([128, 512], F32, tag="pp")
                            for j in range(HCH):
                                nc.tensor.matmul(
                                    pp[:],
                                    tT_t[:, j, r * 128 : (r + 1) * 128],
                                    w_t[:, j, :],
                                    start=(j == 0),
                                    stop=(j == HCH - 1),
                                )
                            nc.vector.tensor_add(vpsh_t[:, r, :], pp[:], bvb_t[:])
                        nc.sync.dma_start(
                            out=vp_sh.rearrange("(r p) o -> p r o", p=128),
                            in_=vpsh_t[:],
                        )
                        nc.gpsimd.collective_compute(
                            "AllGather",
                            mybir.AluOpType.bypass,
                            replica_groups=[list(range(C))],
                            ins=[vp_sh[:].opt()],
                            outs=[vp_g[:].opt()],
                        )


            # ---------------- Phase C/D: attention ----------------------
            with (
                tc.tile_pool(name="kres", bufs=1) as kresp,
                tc.tile_pool(name="vres", bufs=1) as vresp,
                tc.tile_pool(name="expw", bufs=2) as expwp,
                tc.tile_pool(name="wstage", bufs=2) as wstp,
                tc.tile_pool(name="estage", bufs=3) as estp,
                tc.tile_pool(name="stats", bufs=2) as statp,
                tc.tile_pool(name="osmall", bufs=1) as osp,
                tc.tile_pool(name="ps_w", bufs=2, space="PSUM") as ps_w,
                tc.tile_pool(name="ps_t", bufs=1, space="PSUM") as ps_t,
                tc.tile_pool(name="ps_o", bufs=1, space="PSUM") as ps_o,
                tc.tile_pool(name="ps_m", bufs=1, space="PSUM") as ps_m,
            ):
                kres_t = kresp.tile([128, HCH, N], FP16)
                for c in range(C):
                    nc.sync.dma_start(
                        out=kres_t[:, :, c * NL : (c + 1) * NL],
                        in_=kpt_g[c].rearrange("(j p) s -> p j s", p=128),
                    )
                vres_t = vresp.tile([128, N // 128, H], BF16)
                for c in range(C):
                    nc.sync.dma_start(
                        out=vres_t[:, c * (NL // 128) : (c + 1) * (NL // 128), :],
                        in_=vp_g[c * NL : (c + 1) * NL].rearrange(
                            "(sc p) o -> p sc o", p=128
                        ),
                    )

                NSP = 8  # 1024-wide s superblocks
                for i in range(NCH):
                    expw_t = expwp.tile([128, N], BF16, tag="expw")
                    rs_t = statp.tile([128, NSP], F32, tag="rs")
                    po = ps_o.tile([128, H], F32, tag="po")
                    for sp in range(NSP):
                        pw = ps_w.tile([128, 1024], F32, tag="pw")
                        for h in range(2):
                            for j in range(HCH):
                                nc.tensor.matmul(
                                    pw[:, h * 512 : (h + 1) * 512],
                                    qpt_t[:, j, i * 128 : (i + 1) * 128],
                                    kres_t[:, j, sp * 1024 + h * 512 : sp * 1024 + (h + 1) * 512],
                                    start=(j == 0),
                                    stop=(j == HCH - 1),
                                )
                        nc.scalar.activation(
                            expw_t[:, sp * 1024 : (sp + 1) * 1024],
                            pw[:],
                            mybir.ActivationFunctionType.Exp,
                            accum_out=rs_t[:, sp : sp + 1],
                        )
                        pt = ps_t.tile([128, 1024], BF16, tag="pt")
                        for sc in range(8):
                            nc.tensor.transpose(
                                pt[:, sc * 128 : (sc + 1) * 128],
                                expw_t[:, sp * 1024 + sc * 128 : sp * 1024 + (sc + 1) * 128],
                                ident_bf16[:],
                            )
                        et = estp.tile([128, 1024], BF16, tag="et")
                        nc.vector.tensor_copy(et[:], pt[:])
                        for sc in range(8):
                            nc.tensor.matmul(
                                po[:],
                                et[:, sc * 128 : (sc + 1) * 128],
                                vres_t[:, sp * 8 + sc, :],
                                start=(sp == 0 and sc == 0),
                                stop=(sp == NSP - 1 and sc == 7),
                            )
                    rsum_t = statp.tile([128, 1], F32, tag="rsum")
                    nc.vector.reduce_sum(
                        rsum_t[:], rs_t[:], axis=mybir.AxisListType.X
                    )
                    recip_t = statp.tile([128, 1], F32, tag="recip")
                    nc.vector.reciprocal(recip_t[:], rsum_t[:])

                    # normalized rows out (overlaps next chunk's compute)
                    for wb in range(8):
                        wst = wstp.tile([128, 1024], F32, tag="wst")
                        if wb % 2 == 0:
                            nc.vector.tensor_scalar_mul(
                                wst[:],
                                expw_t[:, wb * 1024 : (wb + 1) * 1024],
                                recip_t[:],
                            )
                        else:
                            nc.scalar.activation(
                                wst[:],
                                expw_t[:, wb * 1024 : (wb + 1) * 1024],
                                mybir.ActivationFunctionType.Copy,
                                scale=recip_t[:],
                            )
                        nc.sync.dma_start(
                            out=norm_o[
                                i * 128 : (i + 1) * 128, wb * 1024 : (wb + 1) * 1024
                            ],
                            in_=wst[:],
                        )

                    # out rows: scale, transpose, project, bias
                    osc_t = osp.tile([128, H], BF16, tag="osc")
                    nc.scalar.activation(
                        osc_t[:],
                        po[:],
                        mybir.ActivationFunctionType.Copy,
                        scale=recip_t[:],
                    )
                    pt2 = ps_m.tile([128, HCH, 128], BF16, tag="pt2")
                    for j in range(HCH):
                        nc.tensor.transpose(
                            pt2[:, j, :],
                            osc_t[:, j * 128 : (j + 1) * 128],
                            ident_bf16[:],
                        )
                    oT_t = osp.tile([128, HCH, 128], BF16, tag="oT")
                    nc.vector.tensor_copy(oT_t[:], pt2[:])
                    pf = ps_m.tile([128, H], F32, tag="pf")
                    for j in range(HCH):
                        nc.tensor.matmul(
                            pf[:],
                            oT_t[:, j, :],
                            wot_t[:, j, :],
                            start=(j == 0),
                            stop=(j == HCH - 1),
                        )
                    of_t = osp.tile([128, H], F32, tag="of")
                    nc.vector.tensor_add(of_t[:], pf[:], bob_t[:])
                    nc.sync.dma_start(
                        out=out_o[i * 128 : (i + 1) * 128, :], in_=of_t[:]
                    )

    nc.compile()
    return nc


_NC = None


def kernel(q, k, v, Wq, bq, Wk, bk, Wv, bv, Wo, bo):
    global _NC, LAST_RESULT
    q = np.ascontiguousarray(np.asarray(q, dtype=np.float32))
    k = np.ascontiguousarray(np.asarray(k, dtype=np.float32))
    v = np.ascontiguousarray(np.asarray(v, dtype=np.float32))
    Wq = np.asarray(Wq, dtype=np.float32)
    Wk = np.asarray(Wk, dtype=np.float32)
    Wv = np.asarray(Wv, dtype=np.float32)
    Wo = np.asarray(Wo, dtype=np.float32)
    bq = np.asarray(bq, dtype=np.float32)
    bk = np.asarray(bk, dtype=np.float32)
    bv = np.asarray(bv, dtype=np.float32)
    bo = np.asarray(bo, dtype=np.float32)

    if _NC is None:
        _NC = _build()

    wqt = np.ascontiguousarray(Wq.T)
    wkt = np.ascontiguousarray(Wk.T)
    wvt = np.ascontiguousarray(Wv.T)
    wot = np.ascontiguousarray(Wo.T)
    bq_r = np.ascontiguousarray(bq.reshape(HCH, 128, 1))
    bk_r = np.ascontiguousarray(bk.reshape(HCH, 128, 1))
    bvb = np.ascontiguousarray(np.broadcast_to(bv[None, :], (128, H)))
    bob = np.ascontiguousarray(np.broadcast_to(bo[None, :], (128, H)))

    in_maps = []
    for c in range(C):
        in_maps.append(
            {
                "q": q[c * NL : (c + 1) * NL],
                "k": k[c * NL : (c + 1) * NL],
                "v": v[c * NL : (c + 1) * NL],
                "wqt": wqt,
                "wkt": wkt,
                "wvt": wvt,
                "wot": wot,
                "bq": bq_r,
                "bk": bk_r,
                "bvb": bvb,
                "bob": bob,
            }
        )

    res = run_bass_kernel_spmd(_NC, in_maps, core_ids=list(range(C)))
    LAST_RESULT = res
    normalized = np.concatenate(
        [res.results[c]["norm"] for c in range(C)], axis=0
    )
    out = np.concatenate([res.results[c]["o"] for c in range(C)], axis=0)
    return (out, normalized)
